# revision 15
# baseline (speedup 1.0000x reference)
"""Trainium2 Bass kernel for segmented attention — v4.

Key ideas vs v3:
  - ACT (exp) is the roofline engine (~151us floor). Everything is
    restructured so ACT runs a continuous stream of 1536-wide
    activations: per-seg PSUM score tiles [128,1536] (3 banks) holding
    3 key-chunks, ACT drains each in one call.
  - Deep cross-half software pipelining: pv/normalize/out-projection of
    half X and the q/k/v projections of the next batch are emitted as
    PE "filler" between the score slots of half X+1, so the PE never
    idles long enough for HAM to re-throttle (v3 lost ~100us to K=4/8).
  - Wave pairs with disjoint PE row strips ({3,5},{4,6},{2,1},{7,0})
    keep score matmuls concurrent; seg 0 moved to pb=96 to pair with 7.
  - The reciprocal broadcast (K=9 matmuls in v3) moved to gpsimd
    partition_broadcast; normalize mul is now bf16 SBUF*SBUF (DVE 2x).
  - PSUM: 2x[128,1536] scores + 1x[128,512] pv-accum + 1x[128,512]
    shared proj/out = exactly 8 banks.
"""

import os
import math
import numpy as np
from collections import deque
from contextlib import ExitStack

import concourse.bacc as bacc
import concourse.tile as tile
import concourse.mybir as mybir
from concourse.bass_utils import run_bass_kernel_spmd

F32 = mybir.dt.float32
BF16 = mybir.dt.bfloat16
AF = mybir.ActivationFunctionType

HID = 441
HIDA = HID + 1  # +1 ones row for bias folding
HID2 = HID + 1  # Wo free-dim pad to even
S = 1024
SH = 512
B = 16
N_CORES = 8
BPC = B // N_CORES
BOUNDS = [0, 7, 21, 49, 105, 161, 217, 273, 357, 441]
NSEG = 9
DSEG = [BOUNDS[i + 1] - BOUNDS[i] for i in range(NSEG)]
NHC = 4
HCH_IN = [(i * 128, min(128, HIDA - i * 128)) for i in range(NHC)]  # 442 rows
HCH_OUT = [(i * 128, min(128, HID - i * 128)) for i in range(NHC)]  # 441 rows
NTC = 8
NPT = 5

# ---- scores-side packing of q/k rows: whole segments ----
# seg -> (pack_tile, base). Units pair segments with disjoint strips.
SC_PACK = {
    7: (0, 0),
    8: (1, 0),
    3: (2, 0),
    5: (2, 64),
    4: (3, 0),
    6: (3, 64),
    2: (4, 0),
    1: (4, 32),
    0: (4, 96),
}
# Per-half slot stream: units of 1-2 segs; within a unit the two segs
# alternate slots of 3/3/2 key-chunks. Disjoint PE row strips per unit.
UNITS = [[8], [3, 5], [4, 6], [2, 1], [7, 0]]
CHUNK_SPLITS = [(0, 3), (3, 3), (6, 2)]  # (first chunk, n chunks)

# ---- ctx-side packing: pieces (name, seg, src_off, ln, pt, pb, has_den) ----
PIECES = [
    ("A7", 7, 0, 56, 0, 0, False),
    ("s5", 5, 0, 56, 0, 64, True),
    ("A8", 8, 0, 56, 1, 0, False),
    ("s6", 6, 0, 56, 1, 64, True),
    ("s3", 3, 0, 56, 2, 0, True),
    ("s4", 4, 0, 56, 2, 64, True),
    ("B7", 7, 56, 28, 3, 0, True),
    ("B8", 8, 56, 28, 3, 64, True),
    ("s2", 2, 0, 28, 3, 96, True),
    ("s1", 1, 0, 14, 4, 64, True),
    ("s0", 0, 0, 7, 4, 0, True),
]
PBYN = {p[0]: p for p in PIECES}
# col-strip base 32 is avoided everywhere: matmuls with tile_position
# (0, 32) produce garbage on this hardware (col quadrant 1 bug).
PV_TILES = [["A7", "s5"], ["A8", "s6"], ["s3", "s4"], ["B7", "B8", "s2"], ["s0", "s1"]]
# pv groups ready within their own half (all source segs in units 0-2):
PV_IN_HALF = [2, 1]  # {s3,s4}, {A8,s6} — E ready after slot 14
PV_NEXT_HALF = [0, 3, 4]  # {A7,s5}, {B7,B8,s2}, {s0,s1} — need E7/E0/E1

AUG_OFF = [BOUNDS[i] + i for i in range(NSEG)]
AUG_W = HID + NSEG  # 450
AUG_WP = 512  # zero-padded so widened pv reads stay in bounds
# pv matmul widths, extended over each tile's pad rows so those rows get
# finite junk instead of stale PSUM NaNs; the full-K out-projection can
# then safely read all 128 rows of cxT (junk rows hit zero Wo rows, and
# the recb broadcast covers all 128 rows with finite reciprocals).
PV_W = {"A7": 64, "s5": 64, "A8": 64, "s6": 64, "s3": 64, "s4": 64,
        "B7": 64, "B8": 32, "s2": 32, "s0": 64, "s1": 64}

_CACHE = {}


def _build():
    nc = bacc.Bacc("TRN2", target_bir_lowering=False, debug=False)

    hsT = nc.dram_tensor("hsT", [BPC, HIDA, S], BF16, kind="ExternalInput").ap()
    Wqp_d = nc.dram_tensor("Wqp", [HIDA, NPT * 128], BF16, kind="ExternalInput").ap()
    Wkp_d = nc.dram_tensor("Wkp", [HIDA, NPT * 128], BF16, kind="ExternalInput").ap()
    Wva_d = nc.dram_tensor("Wva", [HIDA, AUG_WP], BF16, kind="ExternalInput").ap()
    Wop_d = nc.dram_tensor("Wop", [NPT, 128, HID2], BF16, kind="ExternalInput").ap()
    indp_d = nc.dram_tensor("indp", [NPT, NSEG, 128], BF16, kind="ExternalInput").ap()
    outT = nc.dram_tensor("outT", [BPC, HID, S], F32, kind="ExternalOutput").ap()

    with tile.TileContext(nc) as tc, ExitStack() as ctx, nc.allow_low_precision(
        reason="bf16 matmuls + bf16 softmax intermediates"
    ):
        cpool = ctx.enter_context(tc.tile_pool(name="c", bufs=1))
        hpool = ctx.enter_context(tc.tile_pool(name="h", bufs=1))
        kpool = ctx.enter_context(tc.tile_pool(name="k", bufs=1))
        qpool = ctx.enter_context(tc.tile_pool(name="q", bufs=1))
        vpool = ctx.enter_context(tc.tile_pool(name="v", bufs=1))
        epool = ctx.enter_context(tc.tile_pool(name="e", bufs=1))
        upool = ctx.enter_context(tc.tile_pool(name="u", bufs=1))
        dpool = ctx.enter_context(tc.tile_pool(name="d", bufs=1))
        rpool = ctx.enter_context(tc.tile_pool(name="r", bufs=1))
        cxpool = ctx.enter_context(tc.tile_pool(name="cx", bufs=1))
        opool = ctx.enter_context(tc.tile_pool(name="o", bufs=2))
        ps_sc = ctx.enter_context(tc.tile_pool(name="psc", bufs=2, space="PSUM"))
        ps_pu = ctx.enter_context(tc.tile_pool(name="ppu", bufs=1, space="PSUM"))
        ps_x = ctx.enter_context(tc.tile_pool(name="px", bufs=1, space="PSUM"))

        # ---- constants, spread across DMA queues for parallel startup ----
        Wq_sb, Wk_sb, Wv_sb = [], [], []
        for hc, (h0, hw) in enumerate(HCH_IN):
            t = cpool.tile([hw, NPT * 128], BF16, name=f"wq{hc}", tag=f"wq{hc}")
            nc.gpsimd.dma_start(out=t, in_=Wqp_d[h0 : h0 + hw, :])
            Wq_sb.append(t)
            t = cpool.tile([hw, NPT * 128], BF16, name=f"wk{hc}", tag=f"wk{hc}")
            nc.scalar.dma_start(out=t, in_=Wkp_d[h0 : h0 + hw, :])
            Wk_sb.append(t)
            t = cpool.tile([hw, AUG_WP], BF16, name=f"wv{hc}", tag=f"wv{hc}")
            nc.gpsimd.dma_start(out=t, in_=Wva_d[h0 : h0 + hw, :])
            Wv_sb.append(t)
        Wo_sb, ind_sb = [], []
        for i in range(NPT):
            t = cpool.tile([128, HID2], BF16, name=f"wo{i}", tag=f"wo{i}")
            nc.scalar.dma_start(out=t, in_=Wop_d[i])
            Wo_sb.append(t)
            t = cpool.tile([NSEG, 128], BF16, name=f"ind{i}", tag=f"ind{i}")
            nc.gpsimd.dma_start(out=t, in_=indp_d[i])
            ind_sb.append(t)

        # ================= emission machinery =================
        # Work items: (min_slot_key, est_ns, closure). min_slot_key is a
        # global slot counter value before which the item may not be
        # emitted (to respect E-readiness within a half).
        work = deque()
        slot_counter = [0]

        def push(closure, est_ns, min_slot=-1):
            work.append((min_slot, est_ns, closure))

        def drain(budget_ns):
            spent = 0
            deferred = []
            while work and spent < budget_ns:
                min_slot, est, fn = work[0]
                if min_slot > slot_counter[0]:
                    # blocked item: look no further (keep order stable)
                    break
                work.popleft()
                fn()
                spent += est
            return spent

        def drain_all():
            while work:
                _, _, fn = work.popleft()
                fn()

        # ---- building blocks ----
        hs_all = {}

        def load_hs(b):
            hs = []
            for hc, (h0, hw) in enumerate(HCH_IN):
                t = hpool.tile([hw, S], BF16, name=f"hs{hc}", tag=f"hs{hc}", bufs=2)
                nc.sync.dma_start(out=t, in_=hsT[b, h0 : h0 + hw, :])
                hs.append(t)
            hs_all[b] = hs

        kT_all = {}  # kT_all[b][pt] — full-S key tiles
        qT_all = {}  # qT_all[(b, half)][pt] — per-half query tiles

        def emit_kproj(b, pt):
            if b not in kT_all:
                kT_all[b] = [None] * NPT
            hs = hs_all[b]
            kt = kpool.tile([128, S], BF16, name=f"kT{pt}", tag=f"kT{pt}", bufs=2)
            for half in range(2):
                pa = ps_x.tile([128, SH], F32, name=f"pk{pt}{half}", tag="x")
                for hc, (h0, hw) in enumerate(HCH_IN):
                    nc.tensor.matmul(
                        pa[:],
                        Wk_sb[hc][:, pt * 128 : (pt + 1) * 128],
                        hs[hc][:, half * SH : (half + 1) * SH],
                        start=(hc == 0),
                        stop=(hc == NHC - 1),
                    )
                nc.vector.tensor_copy(kt[:, half * SH : (half + 1) * SH], pa[:])
            kT_all[b][pt] = kt

        def emit_qproj(b, half, pt):
            key = (b, half)
            if key not in qT_all:
                qT_all[key] = [None] * NPT
            hs = hs_all[b]
            qt = qpool.tile([128, SH], BF16, name=f"qT{pt}", tag=f"qT{pt}", bufs=2)
            pa = ps_x.tile([128, SH], F32, name=f"pq{pt}{half}", tag="x")
            for hc, (h0, hw) in enumerate(HCH_IN):
                nc.tensor.matmul(
                    pa[:],
                    Wq_sb[hc][:, pt * 128 : (pt + 1) * 128],
                    hs[hc][:, half * SH : (half + 1) * SH],
                    start=(hc == 0),
                    stop=(hc == NHC - 1),
                )
            nc.vector.tensor_copy(qt[:], pa[:])
            qT_all[key][pt] = qt

        va_all = {}

        def emit_vproj(b, sc):
            if b not in va_all:
                va_all[b] = [None] * NTC
            hs = hs_all[b]
            pv = ps_x.tile([128, AUG_WP], F32, name=f"pv{sc}", tag="x")
            for hc, (h0, hw) in enumerate(HCH_IN):
                nc.tensor.matmul(
                    pv[:],
                    hs[hc][:, sc * 128 : (sc + 1) * 128],
                    Wv_sb[hc][:],
                    start=(hc == 0),
                    stop=(hc == NHC - 1),
                )
            va = vpool.tile([128, AUG_WP], BF16, name=f"va{sc}", tag=f"va{sc}", bufs=2)
            nc.vector.tensor_copy(va[:], pv[:])
            va_all[b][sc] = va

        # ---- per-half state ----
        halves = [(0, 0), (0, 1), (1, 0), (1, 1)]
        E_all = {}  # (b, half) -> [E tiles per seg]
        u_all = {}  # (b, half) -> [u tiles per pack tile]
        den_all = {}  # (b, half) -> den9 tile
        cx_all = {}  # (b, half) -> [cx tiles per pack tile]

        def emit_scores_slot(b, half, seg, c0, nch):
            """One score slot: nch key-chunk matmuls + one ACT call."""
            X = (b, half)
            if X not in E_all:
                E_all[X] = [None] * NSEG
            if E_all[X][seg] is None:
                # segs 5 and 8 double-buffered: their pv readers run as
                # cross-half filler, so the next half's writes would stall
                # on a single buffer right at a unit boundary.
                bufs = 2 if seg in (5, 8) else 1
                E_all[X][seg] = epool.tile(
                    [128, NTC * SH], BF16, name=f"E{seg}", tag=f"E{seg}", bufs=bufs
                )
            E = E_all[X][seg]
            pt, pb = SC_PACK[seg]
            d = DSEG[seg]
            kt = kT_all[b][pt]
            qt = qT_all[X][pt]
            w = nch * SH
            pms = ps_sc.tile([128, 3 * SH], F32, name=f"pm{seg}{c0}", tag="sc")
            for k in range(nch):
                t = c0 + k
                nc.tensor.matmul(
                    pms[:, k * SH : (k + 1) * SH],
                    kt[pb : pb + d, t * 128 : (t + 1) * 128],
                    qt[pb : pb + d, :],
                    start=True,
                    stop=True,
                    tile_position=(pb, 0),
                )
            nc.scalar.activation(
                E[:, c0 * SH : c0 * SH + w],
                pms[:, 0:w],
                AF.Exp,
                scale=1.0 / math.sqrt(d),
            )

        def push_pv_group(X, g, min_slot):
            """Queue one pv accumulation group (8 t-chunk bundles)."""
            b, half = X
            tiles = PV_TILES[g]
            state = {}

            def start():
                state["pu"] = ps_pu.tile([128, SH], F32, name=f"pu{g}", tag="pu")

            def chunk(t):
                pu = state["pu"]
                E = E_all[X]
                vaug = va_all[b]
                for pn in tiles:
                    _, seg, off, ln, _, pb, has_den = PBYN[pn]
                    a0 = AUG_OFF[seg] + off
                    w = PV_W[pn]
                    nc.tensor.matmul(
                        pu[pb : pb + w, :],
                        vaug[t][:, a0 : a0 + w],
                        E[seg][:, t * SH : (t + 1) * SH],
                        start=(t == 0),
                        stop=(t == NTC - 1),
                        tile_position=(0, pb),
                        skip_group_check=True,
                    )

            def finish():
                pu = state["pu"]
                if X not in u_all:
                    u_all[X] = [None] * NPT
                u = upool.tile([128, SH], BF16, name=f"u{g}", tag=f"u{g}")
                nc.vector.tensor_copy(u[:], pu[:])
                u_all[X][g] = u
                if X not in den_all:
                    den_all[X] = dpool.tile([NSEG, SH], F32, name="den9", tag="den9")
                den9 = den_all[X]
                for pn in tiles:
                    _, seg, off, ln, _, pb, has_den = PBYN[pn]
                    if has_den:
                        nc.gpsimd.dma_start(
                            out=den9[seg : seg + 1, :], in_=u[pb + ln : pb + ln + 1, :]
                        )

            def mk(t):
                def fn():
                    if t == 0:
                        start()
                    chunk(t)
                    if t == NTC - 1:
                        finish()
                return fn

            for t in range(NTC):
                push(mk(t), 320 if t < NTC - 1 else 450, min_slot)

        def push_norm(X, min_slot):
            b, half = X
            state = {}

            def recip():
                den9 = den_all[X]
                rec9 = rpool.tile([NSEG, SH], F32, name="rec9", tag="rec9")
                scr9 = rpool.tile([NSEG, SH], F32, name="scr9", tag="scr9")
                nc.vector.reciprocal_approx_accurate(rec9[:], den9[:], scratch=scr9[:])
                rec9b = rpool.tile([NSEG, SH], BF16, name="rec9b", tag="rec9b")
                nc.vector.tensor_copy(rec9b[:], rec9[:])
                state["rec9b"] = rec9b
                cx_all[X] = [None] * NPT

            def mk(pt):
                def fn():
                    if pt == 0:
                        recip()
                    # piecewise broadcast of reciprocals via indicator
                    # matmul into the pv PSUM bank (free between groups)
                    rb = ps_pu.tile([128, SH], F32, name=f"rb{pt}", tag="pu")
                    nc.tensor.matmul(
                        rb[:], ind_sb[pt][:], state["rec9b"][:],
                        start=True, stop=True,
                    )
                    cx = cxpool.tile([128, SH], BF16, name=f"cx{pt}", tag=f"cx{pt}")
                    nc.vector.tensor_mul(cx[:], u_all[X][pt][:], rb[:])
                    cx_all[X][pt] = cx

                return fn

            for pt in range(NPT):
                push(mk(pt), 500, min_slot)

        def push_wo(X, min_slot):
            b, half = X
            hsl = slice(half * SH, (half + 1) * SH)

            def mk(hc):
                h0, hw = HCH_OUT[hc]

                def fn():
                    cxT = cx_all[X]
                    po = ps_x.tile([128, SH], F32, name=f"po{hc}", tag="x")
                    for pt in range(NPT):
                        nc.tensor.matmul(
                            po[0:hw, :],
                            Wo_sb[pt][:, h0 : h0 + hw],
                            cxT[pt][:],
                            start=(pt == 0),
                            stop=(pt == NPT - 1),
                        )
                    osb = opool.tile([128, SH], F32, name=f"osb{hc}", tag="osb")
                    nc.vector.tensor_copy(osb[0:hw, :], po[0:hw, :])
                    nc.sync.dma_start(out=outT[b, h0 : h0 + hw, hsl], in_=osb[0:hw, :])

                return fn

            for hc in range(NHC):
                push(mk(hc), 1150, min_slot)

        def emit_half(b, half):
            """Emit the 27 score slots of one half, draining filler work
            between slots."""
            X = (b, half)
            base = slot_counter[0]
            for unit in UNITS:
                for c0, nch in CHUNK_SPLITS:
                    for seg in unit:
                        emit_scores_slot(b, half, seg, c0, nch)
                        slot_counter[0] += 1
                        # ACT time for this slot minus the slot's own MMs
                        drain(nch * 430 - 220)
            return base

        # ================= schedule =================
        load_hs(0)
        # head: just enough projection for the first unit ({8} -> pt1)
        emit_kproj(0, 1)
        emit_qproj(0, 0, 1)

        def push_proj_batch0_rest():
            # remaining k/q for b0h0 in unit order
            for pt in [2, 3, 4, 0]:
                push(lambda pt=pt: emit_kproj(0, pt), 1900)
                push(lambda pt=pt: emit_qproj(0, 0, pt), 950)

        push_proj_batch0_rest()

        for i, X in enumerate(halves):
            b, half = X
            # window-specific projection fillers (queued AHEAD of this
            # half's in-half pv so they drain first)
            if i == 0:
                # b0h0 fillers: hs(b1), v(b0), q(b0h1), then k(b1) (b0h0
                # has slack — no previous half's pv/wo to absorb)
                push(lambda: load_hs(1), 100)
                for sc in range(NTC):
                    push(lambda sc=sc: emit_vproj(0, sc), 950)
                for pt in range(NPT):
                    push(lambda pt=pt: emit_qproj(0, 1, pt), 950)
                for pt in [1, 2, 3, 4, 0]:
                    push(lambda pt=pt: emit_kproj(1, pt), 1900)
            elif i == 1:
                # b0h1 fillers: q(b1h0), first half of v(b1)
                for pt in [1, 2, 3, 4, 0]:
                    push(lambda pt=pt: emit_qproj(1, 0, pt), 950)
                for sc in range(4):
                    push(lambda sc=sc: emit_vproj(1, sc), 950)
            elif i == 2:
                # b1h0 fillers: rest of v(b1), q(b1h1)
                for sc in range(4, NTC):
                    push(lambda sc=sc: emit_vproj(1, sc), 950)
                for pt in range(NPT):
                    push(lambda pt=pt: emit_qproj(1, 1, pt), 950)

            # within-half pv groups: pushed BEFORE emit_half so they drain
            # during this half's own slots (E ready after unit 2 = slot 15)
            base = slot_counter[0]
            for g in PV_IN_HALF:
                push_pv_group(X, g, base + 15)

            emit_half(b, half)

            # cross-half pv groups + normalize + out-projection: drain as
            # filler during the next half (or the tail flush for the last)
            for g in PV_NEXT_HALF:
                push_pv_group(X, g, base + 27)
            push_norm(X, base + 27)
            push_wo(X, base + 27)

        drain_all()

    nc.compile()
    return nc


import ml_dtypes

BF16NP = ml_dtypes.bfloat16


def _prep_core_inputs(hidden_states, Wq, bq, Wk, bk, Wv, bv, Wo, bo):
    """Host-side layout prep (transpose/reorder/pad only, no math)."""
    f32 = np.float32
    hs = np.ascontiguousarray(hidden_states.astype(f32, copy=False))
    Wq = np.asarray(Wq, dtype=f32)
    Wk = np.asarray(Wk, dtype=f32)
    Wv = np.asarray(Wv, dtype=f32)
    Wo = np.asarray(Wo, dtype=f32)
    bq = np.asarray(bq, dtype=f32)
    bk = np.asarray(bk, dtype=f32)
    bv = np.asarray(bv, dtype=f32)
    bo = np.asarray(bo, dtype=f32)

    # scores-side q/k packing (whole segments)
    Wqp = np.zeros((HIDA, NPT * 128), dtype=f32)
    Wkp = np.zeros((HIDA, NPT * 128), dtype=f32)
    for seg, (pt, pb) in SC_PACK.items():
        g0, d = BOUNDS[seg], DSEG[seg]
        Wqp[:HID, pt * 128 + pb : pt * 128 + pb + d] = Wq[:, g0 : g0 + d]
        Wqp[HID, pt * 128 + pb : pt * 128 + pb + d] = bq[g0 : g0 + d]
        Wkp[:HID, pt * 128 + pb : pt * 128 + pb + d] = Wk[:, g0 : g0 + d]
        Wkp[HID, pt * 128 + pb : pt * 128 + pb + d] = bk[g0 : g0 + d]

    # ctx-side packing (split pieces)
    Wop = np.zeros((NPT, 128, HID2), dtype=BF16NP)
    indp = np.zeros((NPT, NSEG, 128), dtype=BF16NP)
    for pn, seg, off, ln, pt, pb, has_den in PIECES:
        g0 = BOUNDS[seg] + off
        Wop[pt, pb : pb + ln, :HID] = Wo[g0 : g0 + ln, :].astype(BF16NP)
        indp[pt, seg, pb : pb + ln + (1 if has_den else 0)] = 1.0
    Wop[4, 7, :HID] = bo.astype(BF16NP)  # rides on cxT's ~1.0 denom row

    Wva = np.zeros((HIDA, AUG_WP), dtype=f32)
    for sg in range(NSEG):
        s0, s1 = BOUNDS[sg], BOUNDS[sg + 1]
        a0 = AUG_OFF[sg]
        Wva[:HID, a0 : a0 + (s1 - s0)] = Wv[:, s0:s1]
        Wva[HID, a0 : a0 + (s1 - s0)] = bv[s0:s1]
        Wva[HID, a0 + (s1 - s0)] = 1.0  # ones column for the denominator

    shared = {
        "Wqp": Wqp.astype(BF16NP),
        "Wkp": Wkp.astype(BF16NP),
        "Wva": Wva.astype(BF16NP),
        "Wop": Wop,
        "indp": indp,
    }
    in_maps = []
    for c in range(N_CORES):
        shard = hs[c * BPC : (c + 1) * BPC]
        hsA = np.ones((BPC, HIDA, S), dtype=BF16NP)
        hsA[:, :HID, :] = shard.transpose(0, 2, 1).astype(BF16NP)
        m = dict(shared)
        m["hsT"] = hsA
        in_maps.append(m)
    return in_maps


LAST_RESULTS = None


def kernel(hidden_states, Wq, bq, Wk, bk, Wv, bv, Wo, bo):
    global LAST_RESULTS
    if "nc" not in _CACHE:
        _CACHE["nc"] = _build()
    nc = _CACHE["nc"]
    in_maps = _prep_core_inputs(hidden_states, Wq, bq, Wk, bk, Wv, bv, Wo, bo)
    kwargs = {}
    if os.environ.get("KERNEL_TRACE") == "1":
        kwargs["trace"] = True
        td = os.environ.get("KERNEL_TRACE_DIR")
        if td:
            kwargs["tmpdir"] = td
    res = run_bass_kernel_spmd(nc, in_maps, core_ids=list(range(N_CORES)), **kwargs)
    LAST_RESULTS = res
    out = np.empty((B, S, HID), dtype=np.float32)
    for c in range(N_CORES):
        out[c * BPC : (c + 1) * BPC] = res.results[c]["outT"].transpose(0, 2, 1)
    return out


# revision 26
# speedup vs baseline: 1.1019x; 1.1019x over previous
"""Trainium2 Bass kernel for segmented attention — v4.

Key ideas vs v3:
  - ACT (exp) is the roofline engine (~151us floor). Everything is
    restructured so ACT runs a continuous stream of 1536-wide
    activations: per-seg PSUM score tiles [128,1536] (3 banks) holding
    3 key-chunks, ACT drains each in one call.
  - Deep cross-half software pipelining: pv/normalize/out-projection of
    half X and the q/k/v projections of the next batch are emitted as
    PE "filler" between the score slots of half X+1, so the PE never
    idles long enough for HAM to re-throttle (v3 lost ~100us to K=4/8).
  - Wave pairs with disjoint PE row strips ({3,5},{4,6},{2,1},{7,0})
    keep score matmuls concurrent; seg 0 moved to pb=96 to pair with 7.
  - The reciprocal broadcast (K=9 matmuls in v3) moved to gpsimd
    partition_broadcast; normalize mul is now bf16 SBUF*SBUF (DVE 2x).
  - PSUM: 2x[128,1536] scores + 1x[128,512] pv-accum + 1x[128,512]
    shared proj/out = exactly 8 banks.
"""

import os
import math
import numpy as np
from collections import deque
from contextlib import ExitStack

import concourse.bacc as bacc
import concourse.tile as tile
import concourse.mybir as mybir
from concourse.bass_utils import run_bass_kernel_spmd

F32 = mybir.dt.float32
BF16 = mybir.dt.bfloat16
AF = mybir.ActivationFunctionType

HID = 441
HIDA = HID + 1  # +1 ones row for bias folding
HID2 = HID + 1  # Wo free-dim pad to even
S = 1024
SH = 512
B = 16
N_CORES = 8
BPC = B // N_CORES
BOUNDS = [0, 7, 21, 49, 105, 161, 217, 273, 357, 441]
NSEG = 9
DSEG = [BOUNDS[i + 1] - BOUNDS[i] for i in range(NSEG)]
NHC = 4
HCH_IN = [(i * 128, min(128, HIDA - i * 128)) for i in range(NHC)]  # 442 rows
HCH_OUT = [(i * 128, min(128, HID - i * 128)) for i in range(NHC)]  # 441 rows
NTC = 8
NPT = 5

# ---- scores-side packing of q/k rows: whole segments ----
# seg -> (pack_tile, base). Units pair segments with disjoint strips.
SC_PACK = {
    7: (0, 0),
    8: (1, 0),
    3: (2, 0),
    5: (2, 64),
    4: (3, 0),
    6: (3, 64),
    2: (4, 0),
    1: (4, 32),
    0: (4, 96),
}
# Per-half slot stream: units of 1-2 segs; within a unit the two segs
# alternate slots of 2 key-chunks each. Disjoint PE row strips per unit.
UNITS_STD = [[8], [3, 5], [4, 6], [2, 1], [7, 0]]
# last half reordered so most pv groups start within the half (tail trim)
UNITS_LAST = [[7, 0], [2, 1], [8], [3, 5], [4, 6]]
CHUNK_SPLITS = [(0, 2), (2, 2), (4, 2), (6, 2)]  # (first chunk, n chunks)

# ---- ctx-side packing: pieces (name, seg, src_off, ln, pt, pb, has_den) ----
# Segments kept whole (no 56+28 split): fewer, larger pv matmuls. The
# 84-row pieces round their PE tile claim to 128 columns, so they sit
# ALONE in their pv tile (a concurrent piece in the same bank races).
PIECES = [
    ("p7", 7, 0, 84, 0, 0, True),
    ("p8", 8, 0, 84, 1, 0, True),
    ("p3", 3, 0, 56, 2, 0, True),
    ("p5", 5, 0, 56, 2, 64, True),
    ("p4", 4, 0, 56, 3, 0, True),
    ("p6", 6, 0, 56, 3, 64, True),
    ("p0", 0, 0, 7, 4, 0, True),
    ("p1", 1, 0, 14, 4, 64, True),
    ("p2", 2, 0, 28, 4, 96, True),
]
PBYN = {p[0]: p for p in PIECES}
# col-strip base 32 is avoided everywhere: matmuls with tile_position
# (0, 32) produce garbage on this hardware (col quadrant 1 bug).
PV_TILES = [["p7"], ["p8"], ["p3", "p5"], ["p4", "p6"], ["p0", "p1", "p2"]]

AUG_OFF = [BOUNDS[i] + i for i in range(NSEG)]
AUG_W = HID + NSEG  # 450
AUG_WP = 512  # zero-padded so widened pv reads stay in bounds
# pv matmul widths, extended over each tile's pad rows so those rows get
# finite junk instead of stale PSUM NaNs; the full-K out-projection can
# then safely read all 128 rows of cxT (junk rows hit zero Wo rows, and
# the recb broadcast covers all 128 rows with finite reciprocals).
PV_W = {"p7": 96, "p8": 96, "p3": 64, "p5": 64,
        "p4": 64, "p6": 64, "p0": 64, "p1": 32, "p2": 32}

_CACHE = {}


def _build():
    nc = bacc.Bacc("TRN2", target_bir_lowering=False, debug=False)

    hsT = nc.dram_tensor("hsT", [BPC, HIDA, S], BF16, kind="ExternalInput").ap()
    Wqp_d = nc.dram_tensor("Wqp", [HIDA, NPT * 128], BF16, kind="ExternalInput").ap()
    Wkp_d = nc.dram_tensor("Wkp", [HIDA, NPT * 128], BF16, kind="ExternalInput").ap()
    Wva_d = nc.dram_tensor("Wva", [HIDA, AUG_WP], BF16, kind="ExternalInput").ap()
    Wop_d = nc.dram_tensor("Wop", [NPT, 128, HID2], BF16, kind="ExternalInput").ap()
    indp_d = nc.dram_tensor("indp", [NPT, NSEG, 128], BF16, kind="ExternalInput").ap()
    outT = nc.dram_tensor("outT", [BPC, HID, S], F32, kind="ExternalOutput").ap()

    with tile.TileContext(nc) as tc, ExitStack() as ctx, nc.allow_low_precision(
        reason="bf16 matmuls + bf16 softmax intermediates"
    ):
        cpool = ctx.enter_context(tc.tile_pool(name="c", bufs=1))
        hpool = ctx.enter_context(tc.tile_pool(name="h", bufs=1))
        kpool = ctx.enter_context(tc.tile_pool(name="k", bufs=1))
        qpool = ctx.enter_context(tc.tile_pool(name="q", bufs=1))
        vpool = ctx.enter_context(tc.tile_pool(name="v", bufs=1))
        epool = ctx.enter_context(tc.tile_pool(name="e", bufs=1))
        upool = ctx.enter_context(tc.tile_pool(name="u", bufs=1))
        dpool = ctx.enter_context(tc.tile_pool(name="d", bufs=1))
        rpool = ctx.enter_context(tc.tile_pool(name="r", bufs=1))
        cxpool = ctx.enter_context(tc.tile_pool(name="cx", bufs=1))
        opool = ctx.enter_context(tc.tile_pool(name="o", bufs=2))
        ps_sc = ctx.enter_context(tc.tile_pool(name="psc", bufs=2, space="PSUM"))
        ps_pu = ctx.enter_context(tc.tile_pool(name="ppu", bufs=2, space="PSUM"))
        ps_x = ctx.enter_context(tc.tile_pool(name="px", bufs=2, space="PSUM"))

        # ---- constants, spread across DMA queues for parallel startup ----
        Wq_sb, Wk_sb, Wv_sb = [], [], []
        for hc, (h0, hw) in enumerate(HCH_IN):
            t = cpool.tile([hw, NPT * 128], BF16, name=f"wq{hc}", tag=f"wq{hc}")
            nc.gpsimd.dma_start(out=t, in_=Wqp_d[h0 : h0 + hw, :])
            Wq_sb.append(t)
            t = cpool.tile([hw, NPT * 128], BF16, name=f"wk{hc}", tag=f"wk{hc}")
            nc.scalar.dma_start(out=t, in_=Wkp_d[h0 : h0 + hw, :])
            Wk_sb.append(t)
            t = cpool.tile([hw, AUG_WP], BF16, name=f"wv{hc}", tag=f"wv{hc}")
            nc.gpsimd.dma_start(out=t, in_=Wva_d[h0 : h0 + hw, :])
            Wv_sb.append(t)
        Wo_sb, ind_sb = [], []
        for i in range(NPT):
            t = cpool.tile([128, HID2], BF16, name=f"wo{i}", tag=f"wo{i}")
            nc.scalar.dma_start(out=t, in_=Wop_d[i])
            Wo_sb.append(t)
            t = cpool.tile([NSEG, 128], BF16, name=f"ind{i}", tag=f"ind{i}")
            nc.gpsimd.dma_start(out=t, in_=indp_d[i])
            ind_sb.append(t)

        # ================= emission machinery =================
        # Work items: (min_slot_key, est_ns, closure). min_slot_key is a
        # global slot counter value before which the item may not be
        # emitted (to respect E-readiness within a half).
        work = deque()
        slot_counter = [0]

        def push(closure, est_ns, min_slot=-1):
            work.append((min_slot, est_ns, closure))

        def drain(budget_ns):
            spent = 0
            deferred = []
            while work and spent < budget_ns:
                min_slot, est, fn = work[0]
                if min_slot > slot_counter[0]:
                    # blocked item: look no further (keep order stable)
                    break
                work.popleft()
                fn()
                spent += est
            return spent

        def drain_all():
            while work:
                _, _, fn = work.popleft()
                fn()

        # ---- building blocks ----
        hs_all = {}

        def load_hs(b):
            hs = []
            for hc, (h0, hw) in enumerate(HCH_IN):
                t = hpool.tile([hw, S], BF16, name=f"hs{hc}", tag=f"hs{hc}", bufs=2)
                nc.sync.dma_start(out=t, in_=hsT[b, h0 : h0 + hw, :])
                hs.append(t)
            hs_all[b] = hs

        kT_all = {}  # kT_all[b][pt] — full-S key tiles
        qT_all = {}  # qT_all[(b, half)][pt] — per-half query tiles

        def emit_kproj(b, pt):
            if b not in kT_all:
                kT_all[b] = [None] * NPT
            hs = hs_all[b]
            kt = kpool.tile([128, S], BF16, name=f"kT{pt}", tag=f"kT{pt}", bufs=2)
            for half in range(2):
                pa = ps_x.tile([128, SH], F32, name=f"pk{pt}{half}", tag="x")
                for hc, (h0, hw) in enumerate(HCH_IN):
                    nc.tensor.matmul(
                        pa[:],
                        Wk_sb[hc][:, pt * 128 : (pt + 1) * 128],
                        hs[hc][:, half * SH : (half + 1) * SH],
                        start=(hc == 0),
                        stop=(hc == NHC - 1),
                    )
                nc.vector.tensor_copy(kt[:, half * SH : (half + 1) * SH], pa[:])
            kT_all[b][pt] = kt

        def emit_qproj(b, half, pt):
            key = (b, half)
            if key not in qT_all:
                qT_all[key] = [None] * NPT
            hs = hs_all[b]
            qt = qpool.tile([128, SH], BF16, name=f"qT{pt}", tag=f"qT{pt}", bufs=2)
            pa = ps_x.tile([128, SH], F32, name=f"pq{pt}{half}", tag="x")
            for hc, (h0, hw) in enumerate(HCH_IN):
                nc.tensor.matmul(
                    pa[:],
                    Wq_sb[hc][:, pt * 128 : (pt + 1) * 128],
                    hs[hc][:, half * SH : (half + 1) * SH],
                    start=(hc == 0),
                    stop=(hc == NHC - 1),
                )
            nc.vector.tensor_copy(qt[:], pa[:])
            qT_all[key][pt] = qt

        va_all = {}

        def emit_vproj(b, sc):
            if b not in va_all:
                va_all[b] = [None] * NTC
            hs = hs_all[b]
            pv = ps_x.tile([128, AUG_WP], F32, name=f"pv{sc}", tag="x")
            for hc, (h0, hw) in enumerate(HCH_IN):
                nc.tensor.matmul(
                    pv[:],
                    hs[hc][:, sc * 128 : (sc + 1) * 128],
                    Wv_sb[hc][:],
                    start=(hc == 0),
                    stop=(hc == NHC - 1),
                )
            va = vpool.tile([128, AUG_WP], BF16, name=f"va{sc}", tag=f"va{sc}", bufs=2)
            nc.vector.tensor_copy(va[:], pv[:])
            va_all[b][sc] = va

        # ---- per-half state ----
        halves = [(0, 0), (0, 1), (1, 0), (1, 1)]
        E_all = {}  # (b, half) -> [E tiles per seg]
        u_all = {}  # (b, half) -> [u tiles per pack tile]
        den_all = {}  # (b, half) -> den9 tile
        cx_all = {}  # (b, half) -> [cx tiles per pack tile]

        def emit_scores_slot(b, half, seg, c0, nch):
            """One score slot: nch key-chunk matmuls + one ACT call."""
            X = (b, half)
            if X not in E_all:
                E_all[X] = [None] * NSEG
            if E_all[X][seg] is None:
                # segs 7/0 double-buffered: their pv readers run as
                # cross-half filler, so the next half's writes would stall
                # on a single buffer right at a unit boundary.
                bufs = 2 if seg in (7, 0) else 1
                E_all[X][seg] = epool.tile(
                    [128, NTC * SH], BF16, name=f"E{seg}", tag=f"E{seg}", bufs=bufs
                )
            E = E_all[X][seg]
            pt, pb = SC_PACK[seg]
            d = DSEG[seg]
            kt = kT_all[b][pt]
            qt = qT_all[X][pt]
            w = nch * SH
            pms = ps_sc.tile([128, 2 * SH], F32, name=f"pm{seg}{c0}", tag="sc")
            for k in range(nch):
                t = c0 + k
                nc.tensor.matmul(
                    pms[:, k * SH : (k + 1) * SH],
                    kt[pb : pb + d, t * 128 : (t + 1) * 128],
                    qt[pb : pb + d, :],
                    start=True,
                    stop=True,
                    tile_position=(pb, 0),
                )
            nc.scalar.activation(
                E[:, c0 * SH : c0 * SH + w],
                pms[:, 0:w],
                AF.Exp,
                scale=1.0 / math.sqrt(d),
            )

        def push_pv_group(X, g, min_slot):
            """Queue one pv accumulation group (8 t-chunk bundles)."""
            b, half = X
            tiles = PV_TILES[g]
            state = {}

            def start():
                state["pu"] = ps_pu.tile([128, SH], F32, name=f"pu{g}", tag="pu")

            def chunk(t):
                pu = state["pu"]
                E = E_all[X]
                vaug = va_all[b]
                for pn in tiles:
                    _, seg, off, ln, _, pb, has_den = PBYN[pn]
                    a0 = AUG_OFF[seg] + off
                    w = PV_W[pn]
                    nc.tensor.matmul(
                        pu[pb : pb + w, :],
                        vaug[t][:, a0 : a0 + w],
                        E[seg][:, t * SH : (t + 1) * SH],
                        start=(t == 0),
                        stop=(t == NTC - 1),
                        tile_position=(0, pb),
                        skip_group_check=True,
                    )

            def finish():
                pu = state["pu"]
                if X not in u_all:
                    u_all[X] = [None] * NPT
                u = upool.tile([128, SH], BF16, name=f"u{g}", tag=f"u{g}")
                nc.vector.tensor_copy(u[:], pu[:])
                u_all[X][g] = u
                if X not in den_all:
                    den_all[X] = dpool.tile([NSEG, SH], F32, name="den9", tag="den9")
                den9 = den_all[X]
                for pn in tiles:
                    _, seg, off, ln, _, pb, has_den = PBYN[pn]
                    if has_den:
                        nc.gpsimd.dma_start(
                            out=den9[seg : seg + 1, :], in_=u[pb + ln : pb + ln + 1, :]
                        )

            def mk(t):
                def fn():
                    if t == 0:
                        start()
                    chunk(t)
                    if t == NTC - 1:
                        finish()
                return fn

            for t in range(NTC):
                push(mk(t), 350 if t < NTC - 1 else 500, min_slot)

        def push_norm(X, min_slot):
            b, half = X
            state = {}

            def recip():
                den9 = den_all[X]
                rec9 = rpool.tile([NSEG, SH], F32, name="rec9", tag="rec9")
                scr9 = rpool.tile([NSEG, SH], F32, name="scr9", tag="scr9")
                nc.vector.reciprocal_approx_accurate(rec9[:], den9[:], scratch=scr9[:])
                rec9b = rpool.tile([NSEG, SH], BF16, name="rec9b", tag="rec9b")
                nc.vector.tensor_copy(rec9b[:], rec9[:])
                state["rec9b"] = rec9b
                cx_all[X] = [None] * NPT

            def mk(pt):
                def fn():
                    if pt == 0:
                        recip()
                    # piecewise broadcast of reciprocals via indicator
                    # matmul into the pv PSUM bank (free between groups)
                    rb = ps_pu.tile([128, SH], F32, name=f"rb{pt}", tag="pu")
                    nc.tensor.matmul(
                        rb[:], ind_sb[pt][:], state["rec9b"][:],
                        start=True, stop=True,
                    )
                    cx = cxpool.tile([128, SH], BF16, name=f"cx{pt}", tag=f"cx{pt}")
                    nc.vector.tensor_mul(cx[:], u_all[X][pt][:], rb[:])
                    cx_all[X][pt] = cx

                return fn

            for pt in range(NPT):
                push(mk(pt), 500, min_slot)

        def push_wo(X, min_slot):
            b, half = X
            hsl = slice(half * SH, (half + 1) * SH)

            def mk(hc):
                h0, hw = HCH_OUT[hc]

                def fn():
                    cxT = cx_all[X]
                    po = ps_x.tile([128, SH], F32, name=f"po{hc}", tag="x")
                    for pt in range(NPT):
                        nc.tensor.matmul(
                            po[0:hw, :],
                            Wo_sb[pt][:, h0 : h0 + hw],
                            cxT[pt][:],
                            start=(pt == 0),
                            stop=(pt == NPT - 1),
                        )
                    osb = opool.tile([128, SH], F32, name=f"osb{hc}", tag="osb")
                    nc.vector.tensor_copy(osb[0:hw, :], po[0:hw, :])
                    nc.sync.dma_start(out=outT[b, h0 : h0 + hw, hsl], in_=osb[0:hw, :])

                return fn

            for hc in range(NHC):
                push(mk(hc), 1150, min_slot)

        def emit_half(b, half, units):
            """Emit the 36 score slots of one half, draining filler work
            between slots."""
            for unit in units:
                for c0, nch in CHUNK_SPLITS:
                    for seg in unit:
                        emit_scores_slot(b, half, seg, c0, nch)
                        slot_counter[0] += 1
                        # ACT time for this slot minus the slot's own MMs
                        drain(680)

        # ================= schedule =================
        load_hs(0)
        # head: just enough projection for the first unit ({8} -> pt1)
        emit_kproj(0, 1)
        emit_qproj(0, 0, 1)

        def push_proj_batch0_rest():
            # remaining k/q for b0h0 in unit order
            for pt in [2, 3, 4, 0]:
                push(lambda pt=pt: emit_kproj(0, pt), 1900)
                push(lambda pt=pt: emit_qproj(0, 0, pt), 950)

        push_proj_batch0_rest()

        for i, X in enumerate(halves):
            b, half = X
            # window-specific projection fillers (queued AHEAD of this
            # half's in-half pv so they drain first)
            if i == 0:
                # b0h0 fillers: hs(b1), v(b0), q(b0h1), then k(b1) (b0h0
                # has slack — no previous half's pv/wo to absorb)
                push(lambda: load_hs(1), 100)
                for sc in range(NTC):
                    push(lambda sc=sc: emit_vproj(0, sc), 950)
                for pt in range(NPT):
                    push(lambda pt=pt: emit_qproj(0, 1, pt), 950)
                for pt in [1, 2, 3, 4, 0]:
                    push(lambda pt=pt: emit_kproj(1, pt), 1900)
            elif i == 1:
                # b0h1 fillers: q(b1h0), first half of v(b1)
                for pt in [1, 2, 3, 4, 0]:
                    push(lambda pt=pt: emit_qproj(1, 0, pt), 950)
                for sc in range(4):
                    push(lambda sc=sc: emit_vproj(1, sc), 950)
            elif i == 2:
                # b1h0 fillers: rest of v(b1), q(b1h1)
                for sc in range(4, NTC):
                    push(lambda sc=sc: emit_vproj(1, sc), 950)
                for pt in range(NPT):
                    push(lambda pt=pt: emit_qproj(1, 1, pt), 950)

            # within-half pv groups: pushed BEFORE emit_half so they drain
            # during this half's own slots, gated on E readiness
            base = slot_counter[0]
            last = i == len(halves) - 1
            if last:
                units = UNITS_LAST
                # units {7,0}@0-7, {2,1}@8-15, {8}@16-19, {3,5}@20-27,
                # {4,6}@28-35: groups 0={p7}, 4={p0,p1,p2}, 1={p8} start
                # within the half; {p3,p5}, {p4,p6} flush in the tail
                in_half = [(0, 8), (4, 16), (1, 20)]
                cross = [2, 3]
            else:
                units = UNITS_STD
                # units {8}@0-3, {3,5}@4-11, {4,6}@12-19, {2,1}@20-27,
                # {7,0}@28-35: groups 1={p8}, 2={p3,p5}, 3={p4,p6} start
                # in-half; {p7}, {p0,p1,p2} cross into the next half
                in_half = [(1, 4), (2, 12), (3, 20)]
                cross = [0, 4]
            for g, off in in_half:
                push_pv_group(X, g, base + off)

            emit_half(b, half, units)

            # cross-half pv groups + normalize + out-projection: drain as
            # filler during the next half (or the tail flush for the last)
            for g in cross:
                push_pv_group(X, g, base + 36)
            push_norm(X, base + 36)
            push_wo(X, base + 36)

        drain_all()

    nc.compile()
    return nc


import ml_dtypes

BF16NP = ml_dtypes.bfloat16


def _prep_core_inputs(hidden_states, Wq, bq, Wk, bk, Wv, bv, Wo, bo):
    """Host-side layout prep (transpose/reorder/pad only, no math)."""
    f32 = np.float32
    hs = np.ascontiguousarray(hidden_states.astype(f32, copy=False))
    Wq = np.asarray(Wq, dtype=f32)
    Wk = np.asarray(Wk, dtype=f32)
    Wv = np.asarray(Wv, dtype=f32)
    Wo = np.asarray(Wo, dtype=f32)
    bq = np.asarray(bq, dtype=f32)
    bk = np.asarray(bk, dtype=f32)
    bv = np.asarray(bv, dtype=f32)
    bo = np.asarray(bo, dtype=f32)

    # scores-side q/k packing (whole segments)
    Wqp = np.zeros((HIDA, NPT * 128), dtype=f32)
    Wkp = np.zeros((HIDA, NPT * 128), dtype=f32)
    for seg, (pt, pb) in SC_PACK.items():
        g0, d = BOUNDS[seg], DSEG[seg]
        Wqp[:HID, pt * 128 + pb : pt * 128 + pb + d] = Wq[:, g0 : g0 + d]
        Wqp[HID, pt * 128 + pb : pt * 128 + pb + d] = bq[g0 : g0 + d]
        Wkp[:HID, pt * 128 + pb : pt * 128 + pb + d] = Wk[:, g0 : g0 + d]
        Wkp[HID, pt * 128 + pb : pt * 128 + pb + d] = bk[g0 : g0 + d]

    # ctx-side packing (split pieces)
    Wop = np.zeros((NPT, 128, HID2), dtype=BF16NP)
    indp = np.zeros((NPT, NSEG, 128), dtype=BF16NP)
    for pn, seg, off, ln, pt, pb, has_den in PIECES:
        g0 = BOUNDS[seg] + off
        Wop[pt, pb : pb + ln, :HID] = Wo[g0 : g0 + ln, :].astype(BF16NP)
        indp[pt, seg, pb : pb + ln + (1 if has_den else 0)] = 1.0
    Wop[4, 7, :HID] = bo.astype(BF16NP)  # rides on cxT's ~1.0 denom row

    Wva = np.zeros((HIDA, AUG_WP), dtype=f32)
    for sg in range(NSEG):
        s0, s1 = BOUNDS[sg], BOUNDS[sg + 1]
        a0 = AUG_OFF[sg]
        Wva[:HID, a0 : a0 + (s1 - s0)] = Wv[:, s0:s1]
        Wva[HID, a0 : a0 + (s1 - s0)] = bv[s0:s1]
        Wva[HID, a0 + (s1 - s0)] = 1.0  # ones column for the denominator

    shared = {
        "Wqp": Wqp.astype(BF16NP),
        "Wkp": Wkp.astype(BF16NP),
        "Wva": Wva.astype(BF16NP),
        "Wop": Wop,
        "indp": indp,
    }
    in_maps = []
    for c in range(N_CORES):
        shard = hs[c * BPC : (c + 1) * BPC]
        hsA = np.ones((BPC, HIDA, S), dtype=BF16NP)
        hsA[:, :HID, :] = shard.transpose(0, 2, 1).astype(BF16NP)
        m = dict(shared)
        m["hsT"] = hsA
        in_maps.append(m)
    return in_maps


LAST_RESULTS = None


def kernel(hidden_states, Wq, bq, Wk, bk, Wv, bv, Wo, bo):
    global LAST_RESULTS
    if "nc" not in _CACHE:
        _CACHE["nc"] = _build()
    nc = _CACHE["nc"]
    in_maps = _prep_core_inputs(hidden_states, Wq, bq, Wk, bk, Wv, bv, Wo, bo)
    kwargs = {}
    if os.environ.get("KERNEL_TRACE") == "1":
        kwargs["trace"] = True
        td = os.environ.get("KERNEL_TRACE_DIR")
        if td:
            kwargs["tmpdir"] = td
    res = run_bass_kernel_spmd(nc, in_maps, core_ids=list(range(N_CORES)), **kwargs)
    LAST_RESULTS = res
    out = np.empty((B, S, HID), dtype=np.float32)
    for c in range(N_CORES):
        out[c * BPC : (c + 1) * BPC] = res.results[c]["outT"].transpose(0, 2, 1)
    return out


# revision 35
# speedup vs baseline: 1.1864x; 1.0767x over previous
"""Trainium2 Bass kernel for segmented attention — v4.

Key ideas vs v3:
  - ACT (exp) is the roofline engine (~151us floor). Everything is
    restructured so ACT runs a continuous stream of 1536-wide
    activations: per-seg PSUM score tiles [128,1536] (3 banks) holding
    3 key-chunks, ACT drains each in one call.
  - Deep cross-half software pipelining: pv/normalize/out-projection of
    half X and the q/k/v projections of the next batch are emitted as
    PE "filler" between the score slots of half X+1, so the PE never
    idles long enough for HAM to re-throttle (v3 lost ~100us to K=4/8).
  - Wave pairs with disjoint PE row strips ({3,5},{4,6},{2,1},{7,0})
    keep score matmuls concurrent; seg 0 moved to pb=96 to pair with 7.
  - The reciprocal broadcast (K=9 matmuls in v3) moved to gpsimd
    partition_broadcast; normalize mul is now bf16 SBUF*SBUF (DVE 2x).
  - PSUM: 2x[128,1536] scores + 1x[128,512] pv-accum + 1x[128,512]
    shared proj/out = exactly 8 banks.
"""

import os
import math
import numpy as np
from collections import deque
from contextlib import ExitStack

import concourse.bacc as bacc
import concourse.tile as tile
import concourse.mybir as mybir
from concourse.bass_utils import run_bass_kernel_spmd

F32 = mybir.dt.float32
BF16 = mybir.dt.bfloat16
AF = mybir.ActivationFunctionType

HID = 441
HIDA = HID + 1  # +1 ones row for bias folding
HID2 = HID + 1  # Wo free-dim pad to even
S = 1024
SH = 512
B = 16
N_CORES = 8
BPC = B // N_CORES
BOUNDS = [0, 7, 21, 49, 105, 161, 217, 273, 357, 441]
NSEG = 9
DSEG = [BOUNDS[i + 1] - BOUNDS[i] for i in range(NSEG)]
NHC = 4
HCH_IN = [(i * 128, min(128, HIDA - i * 128)) for i in range(NHC)]  # 442 rows
HCH_OUT = [(i * 128, min(128, HID - i * 128)) for i in range(NHC)]  # 441 rows
NTC = 8
NPT = 5

# ---- scores-side packing of q/k rows: whole segments ----
# seg -> (pack_tile, base). Units pair segments with disjoint strips.
SC_PACK = {
    7: (0, 0),
    8: (1, 0),
    3: (2, 0),
    5: (2, 64),
    4: (3, 0),
    6: (3, 64),
    2: (4, 0),
    1: (4, 32),
    0: (4, 96),
}
# Per-half slot stream: units of 1-2 segs; within a unit the two segs
# alternate slots of 2 key-chunks each. Disjoint PE row strips per unit.
UNITS_STD = [[8], [3, 5], [4, 6], [2, 1], [7, 0]]
# last half reordered so most pv groups start within the half (tail trim)
UNITS_LAST = [[7, 0], [2, 1], [8], [3, 5], [4, 6]]
CHUNK_SPLITS = [(0, 2), (2, 2), (4, 2), (6, 2)]  # (first chunk, n chunks)

# ---- ctx-side packing: pieces (name, seg, src_off, ln, pt, pb, has_den) ----
# Segments kept whole (no 56+28 split): fewer, larger pv matmuls. The
# 84-row pieces round their PE tile claim to 128 columns, so they sit
# ALONE in their pv tile (a concurrent piece in the same bank races).
PIECES = [
    ("p7", 7, 0, 84, 0, 0, True),
    ("p8", 8, 0, 84, 1, 0, True),
    ("p3", 3, 0, 56, 2, 0, True),
    ("p5", 5, 0, 56, 2, 64, True),
    ("p4", 4, 0, 56, 3, 0, True),
    ("p6", 6, 0, 56, 3, 64, True),
    ("p0", 0, 0, 7, 4, 0, True),
    ("p1", 1, 0, 14, 4, 64, True),
    ("p2", 2, 0, 28, 4, 96, True),
]
PBYN = {p[0]: p for p in PIECES}
# col-strip base 32 is avoided everywhere: matmuls with tile_position
# (0, 32) produce garbage on this hardware (col quadrant 1 bug).
PV_TILES = [["p7"], ["p8"], ["p3", "p5"], ["p4", "p6"], ["p0", "p1", "p2"]]

AUG_OFF = [BOUNDS[i] + i for i in range(NSEG)]
AUG_W = HID + NSEG  # 450
AUG_WP = 512  # zero-padded so widened pv reads stay in bounds
# pv matmul widths, extended over each tile's pad rows so those rows get
# finite junk instead of stale PSUM NaNs; the full-K out-projection can
# then safely read all 128 rows of cxT (junk rows hit zero Wo rows, and
# the recb broadcast covers all 128 rows with finite reciprocals).
PV_W = {"p7": 96, "p8": 96, "p3": 64, "p5": 64,
        "p4": 64, "p6": 64, "p0": 64, "p1": 32, "p2": 32}

_CACHE = {}


def _build():
    nc = bacc.Bacc("TRN2", target_bir_lowering=False, debug=False)

    hsT = nc.dram_tensor("hsT", [BPC, HIDA, S], BF16, kind="ExternalInput").ap()
    Wqp_d = nc.dram_tensor("Wqp", [HIDA, NPT * 128], BF16, kind="ExternalInput").ap()
    Wkp_d = nc.dram_tensor("Wkp", [HIDA, NPT * 128], BF16, kind="ExternalInput").ap()
    Wva_d = nc.dram_tensor("Wva", [HIDA, AUG_WP], BF16, kind="ExternalInput").ap()
    Wop_d = nc.dram_tensor("Wop", [NPT, 128, HID2], BF16, kind="ExternalInput").ap()
    indp_d = nc.dram_tensor("indp", [NPT, NSEG, 128], BF16, kind="ExternalInput").ap()
    outT = nc.dram_tensor("outT", [BPC, HID, S], F32, kind="ExternalOutput").ap()

    with tile.TileContext(nc) as tc, ExitStack() as ctx, nc.allow_low_precision(
        reason="bf16 matmuls + bf16 softmax intermediates"
    ):
        cpool = ctx.enter_context(tc.tile_pool(name="c", bufs=1))
        hpool = ctx.enter_context(tc.tile_pool(name="h", bufs=1))
        kpool = ctx.enter_context(tc.tile_pool(name="k", bufs=1))
        qpool = ctx.enter_context(tc.tile_pool(name="q", bufs=1))
        vpool = ctx.enter_context(tc.tile_pool(name="v", bufs=1))
        epool = ctx.enter_context(tc.tile_pool(name="e", bufs=1))
        upool = ctx.enter_context(tc.tile_pool(name="u", bufs=1))
        dpool = ctx.enter_context(tc.tile_pool(name="d", bufs=1))
        rpool = ctx.enter_context(tc.tile_pool(name="r", bufs=1))
        cxpool = ctx.enter_context(tc.tile_pool(name="cx", bufs=1))
        opool = ctx.enter_context(tc.tile_pool(name="o", bufs=2))
        ps_sc = ctx.enter_context(tc.tile_pool(name="psc", bufs=2, space="PSUM"))
        ps_pu = ctx.enter_context(tc.tile_pool(name="ppu", bufs=2, space="PSUM"))
        ps_x = ctx.enter_context(tc.tile_pool(name="px", bufs=2, space="PSUM"))

        # ---- constants, spread across DMA queues for parallel startup ----
        Wq_sb, Wk_sb, Wv_sb = [], [], []
        for hc, (h0, hw) in enumerate(HCH_IN):
            t = cpool.tile([hw, NPT * 128], BF16, name=f"wq{hc}", tag=f"wq{hc}")
            nc.gpsimd.dma_start(out=t, in_=Wqp_d[h0 : h0 + hw, :])
            Wq_sb.append(t)
            t = cpool.tile([hw, NPT * 128], BF16, name=f"wk{hc}", tag=f"wk{hc}")
            nc.scalar.dma_start(out=t, in_=Wkp_d[h0 : h0 + hw, :])
            Wk_sb.append(t)
            t = cpool.tile([hw, AUG_WP], BF16, name=f"wv{hc}", tag=f"wv{hc}")
            nc.gpsimd.dma_start(out=t, in_=Wva_d[h0 : h0 + hw, :])
            Wv_sb.append(t)
        Wo_sb, ind_sb = [], []
        for i in range(NPT):
            t = cpool.tile([128, HID2], BF16, name=f"wo{i}", tag=f"wo{i}")
            nc.scalar.dma_start(out=t, in_=Wop_d[i])
            Wo_sb.append(t)
            t = cpool.tile([NSEG, 128], BF16, name=f"ind{i}", tag=f"ind{i}")
            nc.gpsimd.dma_start(out=t, in_=indp_d[i])
            ind_sb.append(t)

        # ================= emission machinery =================
        # Work items: (min_slot_key, est_ns, closure). min_slot_key is a
        # global slot counter value before which the item may not be
        # emitted (to respect E-readiness within a half).
        work = deque()
        slot_counter = [0]

        def push(closure, est_ns, min_slot=-1):
            work.append((min_slot, est_ns, closure))

        def drain(budget_ns):
            spent = 0
            skipped = []
            scan = 0
            while work and spent < budget_ns and scan < 24:
                min_slot, est, fn = work.popleft()
                if min_slot > slot_counter[0]:
                    # gated item: skip past it (restored below, order kept)
                    skipped.append((min_slot, est, fn))
                    scan += 1
                    continue
                fn()
                spent += est
            for item in reversed(skipped):
                work.appendleft(item)
            return spent

        def drain_all():
            while work:
                _, _, fn = work.popleft()
                fn()

        # ---- building blocks ----
        hs_all = {}

        def load_hs(b):
            hs = []
            for hc, (h0, hw) in enumerate(HCH_IN):
                t = hpool.tile([hw, S], BF16, name=f"hs{hc}", tag=f"hs{hc}", bufs=2)
                nc.sync.dma_start(out=t, in_=hsT[b, h0 : h0 + hw, :])
                hs.append(t)
            hs_all[b] = hs

        kT_all = {}  # kT_all[b][pt] — full-S key tiles
        qT_all = {}  # qT_all[(b, half)][pt] — per-half query tiles

        def emit_kproj(b, pt):
            if b not in kT_all:
                kT_all[b] = [None] * NPT
            hs = hs_all[b]
            kt = kpool.tile([128, S], BF16, name=f"kT{pt}", tag=f"kT{pt}", bufs=2)
            for half in range(2):
                pa = ps_x.tile([128, SH], F32, name=f"pk{pt}{half}", tag="x")
                for hc, (h0, hw) in enumerate(HCH_IN):
                    nc.tensor.matmul(
                        pa[:],
                        Wk_sb[hc][:, pt * 128 : (pt + 1) * 128],
                        hs[hc][:, half * SH : (half + 1) * SH],
                        start=(hc == 0),
                        stop=(hc == NHC - 1),
                    )
                nc.vector.tensor_copy(kt[:, half * SH : (half + 1) * SH], pa[:])
            kT_all[b][pt] = kt

        def emit_qproj(b, half, pt):
            key = (b, half)
            if key not in qT_all:
                qT_all[key] = [None] * NPT
            hs = hs_all[b]
            qt = qpool.tile([128, SH], BF16, name=f"qT{pt}", tag=f"qT{pt}", bufs=2)
            pa = ps_x.tile([128, SH], F32, name=f"pq{pt}{half}", tag="x")
            for hc, (h0, hw) in enumerate(HCH_IN):
                nc.tensor.matmul(
                    pa[:],
                    Wq_sb[hc][:, pt * 128 : (pt + 1) * 128],
                    hs[hc][:, half * SH : (half + 1) * SH],
                    start=(hc == 0),
                    stop=(hc == NHC - 1),
                )
            nc.vector.tensor_copy(qt[:], pa[:])
            qT_all[key][pt] = qt

        va_all = {}

        def emit_vproj(b, sc):
            if b not in va_all:
                va_all[b] = [None] * NTC
            hs = hs_all[b]
            pv = ps_x.tile([128, AUG_WP], F32, name=f"pv{sc}", tag="x")
            for hc, (h0, hw) in enumerate(HCH_IN):
                nc.tensor.matmul(
                    pv[:],
                    hs[hc][:, sc * 128 : (sc + 1) * 128],
                    Wv_sb[hc][:],
                    start=(hc == 0),
                    stop=(hc == NHC - 1),
                )
            va = vpool.tile([128, AUG_WP], BF16, name=f"va{sc}", tag=f"va{sc}", bufs=2)
            nc.vector.tensor_copy(va[:], pv[:])
            va_all[b][sc] = va

        # ---- per-half state ----
        halves = [(0, 0), (0, 1), (1, 0), (1, 1)]
        E_all = {}  # (b, half) -> [E tiles per seg]
        u_all = {}  # (b, half) -> [u tiles per pack tile]
        den_all = {}  # (b, half) -> den9 tile
        cx_all = {}  # (b, half) -> [cx tiles per pack tile]

        def emit_scores_slot(b, half, seg, c0, nch):
            """One score slot: nch key-chunk matmuls + one ACT call."""
            X = (b, half)
            if X not in E_all:
                E_all[X] = [None] * NSEG
            if E_all[X][seg] is None:
                # segs 7/0 double-buffered: their pv readers run as
                # cross-half filler, so the next half's writes would stall
                # on a single buffer right at a unit boundary.
                bufs = 2 if seg in (7, 0) else 1
                E_all[X][seg] = epool.tile(
                    [128, NTC * SH], BF16, name=f"E{seg}", tag=f"E{seg}", bufs=bufs
                )
            E = E_all[X][seg]
            pt, pb = SC_PACK[seg]
            d = DSEG[seg]
            kt = kT_all[b][pt]
            qt = qT_all[X][pt]
            w = nch * SH
            pms = ps_sc.tile([128, 2 * SH], F32, name=f"pm{seg}{c0}", tag="sc")
            for k in range(nch):
                t = c0 + k
                nc.tensor.matmul(
                    pms[:, k * SH : (k + 1) * SH],
                    kt[pb : pb + d, t * 128 : (t + 1) * 128],
                    qt[pb : pb + d, :],
                    start=True,
                    stop=True,
                    tile_position=(pb, 0),
                )
            nc.scalar.activation(
                E[:, c0 * SH : c0 * SH + w],
                pms[:, 0:w],
                AF.Exp,
                scale=1.0 / math.sqrt(d),
            )

        def push_pv_group(X, g, min_slot):
            """Queue one pv accumulation group (8 t-chunk bundles)."""
            b, half = X
            tiles = PV_TILES[g]
            state = {}

            def start():
                state["pu"] = ps_pu.tile([128, SH], F32, name=f"pu{g}", tag="pu")

            def chunk(t):
                pu = state["pu"]
                E = E_all[X]
                vaug = va_all[b]
                for pn in tiles:
                    _, seg, off, ln, _, pb, has_den = PBYN[pn]
                    a0 = AUG_OFF[seg] + off
                    w = PV_W[pn]
                    nc.tensor.matmul(
                        pu[pb : pb + w, :],
                        vaug[t][:, a0 : a0 + w],
                        E[seg][:, t * SH : (t + 1) * SH],
                        start=(t == 0),
                        stop=(t == NTC - 1),
                        tile_position=(0, pb),
                        skip_group_check=True,
                    )

            def finish():
                pu = state["pu"]
                if X not in u_all:
                    u_all[X] = [None] * NPT
                u = upool.tile([128, SH], BF16, name=f"u{g}", tag=f"u{g}")
                nc.vector.tensor_copy(u[:], pu[:])
                u_all[X][g] = u
                if X not in den_all:
                    den_all[X] = dpool.tile([NSEG, SH], F32, name="den9", tag="den9")
                den9 = den_all[X]
                for pn in tiles:
                    _, seg, off, ln, _, pb, has_den = PBYN[pn]
                    if has_den:
                        nc.gpsimd.dma_start(
                            out=den9[seg : seg + 1, :], in_=u[pb + ln : pb + ln + 1, :]
                        )

            def mk(t):
                def fn():
                    if t == 0:
                        start()
                    chunk(t)
                    if t == NTC - 1:
                        finish()
                return fn

            for t in range(NTC):
                push(mk(t), 300 if t < NTC - 1 else 420, min_slot)

        def push_norm(X, min_slot):
            b, half = X
            state = {}

            def recip():
                den9 = den_all[X]
                rec9 = rpool.tile([NSEG, SH], F32, name="rec9", tag="rec9")
                scr9 = rpool.tile([NSEG, SH], F32, name="scr9", tag="scr9")
                nc.vector.reciprocal_approx_accurate(rec9[:], den9[:], scratch=scr9[:])
                rec9b = rpool.tile([NSEG, SH], BF16, name="rec9b", tag="rec9b")
                nc.vector.tensor_copy(rec9b[:], rec9[:])
                state["rec9b"] = rec9b
                cx_all[X] = [None] * NPT

            def mk(pt):
                def fn():
                    if pt == 0:
                        recip()
                    # piecewise broadcast of reciprocals via indicator
                    # matmul into the pv PSUM bank (free between groups)
                    rb = ps_pu.tile([128, SH], F32, name=f"rb{pt}", tag="pu")
                    nc.tensor.matmul(
                        rb[:], ind_sb[pt][:], state["rec9b"][:],
                        start=True, stop=True,
                    )
                    cx = cxpool.tile([128, SH], BF16, name=f"cx{pt}", tag=f"cx{pt}")
                    nc.vector.tensor_mul(cx[:], u_all[X][pt][:], rb[:])
                    cx_all[X][pt] = cx

                return fn

            # PE cost is one K=9 matmul per tile; the mul is DVE work
            for pt in range(NPT):
                push(mk(pt), 280, min_slot)

        def push_wo(X, min_slot):
            b, half = X
            hsl = slice(half * SH, (half + 1) * SH)

            def mk(hc):
                h0, hw = HCH_OUT[hc]

                def fn():
                    cxT = cx_all[X]
                    po = ps_x.tile([128, SH], F32, name=f"po{hc}", tag="x")
                    for pt in range(NPT):
                        nc.tensor.matmul(
                            po[0:hw, :],
                            Wo_sb[pt][:, h0 : h0 + hw],
                            cxT[pt][:],
                            start=(pt == 0),
                            stop=(pt == NPT - 1),
                        )
                    osb = opool.tile([128, SH], F32, name=f"osb{hc}", tag="osb")
                    nc.vector.tensor_copy(osb[0:hw, :], po[0:hw, :])
                    nc.sync.dma_start(out=outT[b, h0 : h0 + hw, hsl], in_=osb[0:hw, :])

                return fn

            for hc in range(NHC):
                push(mk(hc), 1250, min_slot)

        def emit_half(b, half, units):
            """Emit the 36 score slots of one half, draining filler work
            between slots."""
            for unit in units:
                for c0, nch in CHUNK_SPLITS:
                    for seg in unit:
                        emit_scores_slot(b, half, seg, c0, nch)
                        slot_counter[0] += 1
                        # ACT time for this slot minus the slot's own MMs;
                        # kept small so filler spreads across all 36 slots
                        drain(520)

        # ================= schedule =================
        load_hs(0)
        # head: just enough projection for the first unit ({8} -> pt1)
        emit_kproj(0, 1)
        emit_qproj(0, 0, 1)

        def push_proj_batch0_rest():
            # remaining k/q for b0h0 in unit order
            for pt in [2, 3, 4, 0]:
                push(lambda pt=pt: emit_kproj(0, pt), 2100)
                push(lambda pt=pt: emit_qproj(0, 0, pt), 1050)

        push_proj_batch0_rest()

        for i, X in enumerate(halves):
            b, half = X
            # window-specific projection fillers (queued AHEAD of this
            # half's in-half pv so they drain first)
            w0 = slot_counter[0]
            if i == 0:
                # b0h0 fillers: hs(b1), v(b0) early (in-half pv needs
                # vaug); q(b0h1)/k(b1) pushed into the late-slot region so
                # the queue doesn't run dry there (HAM stays warm)
                push(lambda: load_hs(1), 100)
                for sc in range(NTC):
                    push(lambda sc=sc: emit_vproj(0, sc), 1050)
                for pt in range(NPT):
                    push(lambda pt=pt: emit_qproj(0, 1, pt), 1050, w0 + 14)
                for pt in [1, 2, 3, 4, 0]:
                    push(lambda pt=pt: emit_kproj(1, pt), 2100, w0 + 20)
            elif i == 1:
                # b0h1 fillers: q(b1h0) mid-half, all of v(b1) late
                for pt in [1, 2, 3, 4, 0]:
                    push(lambda pt=pt: emit_qproj(1, 0, pt), 1050, w0 + 14)
                for sc in range(NTC):
                    push(lambda sc=sc: emit_vproj(1, sc), 1050, w0 + 20)
            elif i == 2:
                # b1h0 fillers: q(b1h1) mid-half
                for pt in range(NPT):
                    push(lambda pt=pt: emit_qproj(1, 1, pt), 1050, w0 + 14)

            # within-half pv groups: pushed BEFORE emit_half so they drain
            # during this half's own slots, gated on E readiness
            base = slot_counter[0]
            last = i == len(halves) - 1
            if last:
                units = UNITS_LAST
                # units {7,0}@0-7, {2,1}@8-15, {8}@16-19, {3,5}@20-27,
                # {4,6}@28-35: groups 0={p7}, 4={p0,p1,p2}, 1={p8} start
                # within the half; {p3,p5}, {p4,p6} flush in the tail
                in_half = [(0, 8), (4, 16), (1, 20)]
                cross = [2, 3]
            else:
                units = UNITS_STD
                # units {8}@0-3, {3,5}@4-11, {4,6}@12-19, {2,1}@20-27,
                # {7,0}@28-35: groups 1={p8}, 2={p3,p5}, 3={p4,p6} start
                # in-half; {p7}, {p0,p1,p2} cross into the next half
                in_half = [(1, 4), (2, 12), (3, 20)]
                cross = [0, 4]
            for g, off in in_half:
                push_pv_group(X, g, base + off)

            emit_half(b, half, units)

            # cross-half pv groups + normalize early in the next half;
            # out-projection gated into its late-slot region (or the tail
            # flush for the last half)
            for g in cross:
                push_pv_group(X, g, base + 36)
            push_norm(X, base + 36)
            push_wo(X, base + 36 + 20)

        drain_all()

    nc.compile()
    return nc


import ml_dtypes

BF16NP = ml_dtypes.bfloat16


def _prep_core_inputs(hidden_states, Wq, bq, Wk, bk, Wv, bv, Wo, bo):
    """Host-side layout prep (transpose/reorder/pad only, no math)."""
    f32 = np.float32
    hs = np.ascontiguousarray(hidden_states.astype(f32, copy=False))
    Wq = np.asarray(Wq, dtype=f32)
    Wk = np.asarray(Wk, dtype=f32)
    Wv = np.asarray(Wv, dtype=f32)
    Wo = np.asarray(Wo, dtype=f32)
    bq = np.asarray(bq, dtype=f32)
    bk = np.asarray(bk, dtype=f32)
    bv = np.asarray(bv, dtype=f32)
    bo = np.asarray(bo, dtype=f32)

    # scores-side q/k packing (whole segments)
    Wqp = np.zeros((HIDA, NPT * 128), dtype=f32)
    Wkp = np.zeros((HIDA, NPT * 128), dtype=f32)
    for seg, (pt, pb) in SC_PACK.items():
        g0, d = BOUNDS[seg], DSEG[seg]
        Wqp[:HID, pt * 128 + pb : pt * 128 + pb + d] = Wq[:, g0 : g0 + d]
        Wqp[HID, pt * 128 + pb : pt * 128 + pb + d] = bq[g0 : g0 + d]
        Wkp[:HID, pt * 128 + pb : pt * 128 + pb + d] = Wk[:, g0 : g0 + d]
        Wkp[HID, pt * 128 + pb : pt * 128 + pb + d] = bk[g0 : g0 + d]

    # ctx-side packing (split pieces)
    Wop = np.zeros((NPT, 128, HID2), dtype=BF16NP)
    indp = np.zeros((NPT, NSEG, 128), dtype=BF16NP)
    for pn, seg, off, ln, pt, pb, has_den in PIECES:
        g0 = BOUNDS[seg] + off
        Wop[pt, pb : pb + ln, :HID] = Wo[g0 : g0 + ln, :].astype(BF16NP)
        indp[pt, seg, pb : pb + ln + (1 if has_den else 0)] = 1.0
    Wop[4, 7, :HID] = bo.astype(BF16NP)  # rides on cxT's ~1.0 denom row

    Wva = np.zeros((HIDA, AUG_WP), dtype=f32)
    for sg in range(NSEG):
        s0, s1 = BOUNDS[sg], BOUNDS[sg + 1]
        a0 = AUG_OFF[sg]
        Wva[:HID, a0 : a0 + (s1 - s0)] = Wv[:, s0:s1]
        Wva[HID, a0 : a0 + (s1 - s0)] = bv[s0:s1]
        Wva[HID, a0 + (s1 - s0)] = 1.0  # ones column for the denominator

    shared = {
        "Wqp": Wqp.astype(BF16NP),
        "Wkp": Wkp.astype(BF16NP),
        "Wva": Wva.astype(BF16NP),
        "Wop": Wop,
        "indp": indp,
    }
    in_maps = []
    for c in range(N_CORES):
        shard = hs[c * BPC : (c + 1) * BPC]
        hsA = np.ones((BPC, HIDA, S), dtype=BF16NP)
        hsA[:, :HID, :] = shard.transpose(0, 2, 1).astype(BF16NP)
        m = dict(shared)
        m["hsT"] = hsA
        in_maps.append(m)
    return in_maps


LAST_RESULTS = None


def kernel(hidden_states, Wq, bq, Wk, bk, Wv, bv, Wo, bo):
    global LAST_RESULTS
    if "nc" not in _CACHE:
        _CACHE["nc"] = _build()
    nc = _CACHE["nc"]
    in_maps = _prep_core_inputs(hidden_states, Wq, bq, Wk, bk, Wv, bv, Wo, bo)
    kwargs = {}
    if os.environ.get("KERNEL_TRACE") == "1":
        kwargs["trace"] = True
        td = os.environ.get("KERNEL_TRACE_DIR")
        if td:
            kwargs["tmpdir"] = td
    res = run_bass_kernel_spmd(nc, in_maps, core_ids=list(range(N_CORES)), **kwargs)
    LAST_RESULTS = res
    out = np.empty((B, S, HID), dtype=np.float32)
    for c in range(N_CORES):
        out[c * BPC : (c + 1) * BPC] = res.results[c]["outT"].transpose(0, 2, 1)
    return out
